# revision 21
# baseline (speedup 1.0000x reference)
"""Multi-head attention (B=16, N=1024, E=768, H=8) on 8 Trainium2 NeuronCores.

Sharding: data-parallel over batch (2 batches per core, no collectives).
Per core, one fused Tile kernel:
  - host pre-transposes x -> x^T and pre-permutes the interleaved qkv weights
    (including packing head-pair Q|K features into full 128-row chunks so the
    QK projection runs at 100% PE utilization; pieces are unscrambled into
    per-head Q^T/K^T tiles via staged copies + partition-shift SBUF DMAs)
  - V is produced per batch for all heads, 97 cols per head: a leading ones
    column (so the softmax denominator falls out of the O matmul as row 0)
    plus the 96 V columns
  - S^T = (K^T)^T @ Q^T -> PSUM, Exp on the scalar engine -> SBUF
  - O = V''^T @ exp(S^T) accumulated over key chunks, software-pipelined
    with the S matmuls (each weight load shared by both q-halves)
  - normalize with reciprocal_approx_fast + gpsimd partition_broadcast + one
    fused DVE multiply that also folds the post-softmax 1/sqrt(E) scale
  - output projection from the transposed O layout (contraction sliced to 97
    rows); bias added on DVE, DMA out
Matmuls run in bf16 (PRECISION="fast") or fp32r (="safe", ~11%% slower,
~6x lower error); softmax/accumulation stays fp32.
"""
import sys
import os

for _p in ("/opt/trn_rl_repo", "/root/.axon_site", "/root/.axon_site/_ro/trn_rl_repo"):
    if os.path.isdir(_p) and _p not in sys.path:
        sys.path.append(_p)

import numpy as np

B, N, E, H = 16, 1024, 768, 8
D = E // H            # 96
NCORES = 8
BPC = B // NCORES     # batches per core = 2
EC = E // 128         # 6 E-chunks
TC = N // 128         # 8 token chunks
DP = 128              # padded per-head width in the proj layout
VW = D + 1            # per-head width in the V layout (ones col + 96 V cols)
SCALE = float(1.0 / np.sqrt(np.float32(E)))

# "fast": bf16 activations/weights on the attention path (~0.6% scale absmax)
# "safe": fp32r (tf32-like) everywhere (~0.04% scale absmax), ~8% slower
PRECISION = "fast"

_NC_CACHE = {}


def _build_nc():
    import concourse.bacc as bacc
    import concourse.mybir as mybir
    import concourse.tile as tile

    FP32 = mybir.dt.float32
    FP32R = mybir.dt.float32r
    BF16 = mybir.dt.bfloat16
    DTF = BF16 if PRECISION == "fast" else FP32R
    AF = mybir.ActivationFunctionType
    OP = mybir.AluOpType

    fast = PRECISION == "fast"
    nc = bacc.Bacc("TRN2", target_bir_lowering=False, debug=False, num_devices=NCORES)

    xt = nc.dram_tensor("xt", [BPC, 128, EC * N], DTF, kind="ExternalInput")
    wqk = nc.dram_tensor("wqk", [H // 2, 128, EC * 4 * D], DTF, kind="ExternalInput")
    wv = nc.dram_tensor("wv", [128, EC * H * VW], DTF, kind="ExternalInput")
    vb = nc.dram_tensor("vb", [128, H * VW], FP32, kind="ExternalInput")
    bqk = nc.dram_tensor("bqk", [128, (H // 2) * 3], FP32, kind="ExternalInput")
    pw = nc.dram_tensor("pw", [128, H * E], BF16, kind="ExternalInput")
    pb = nc.dram_tensor("pb", [128, E], FP32, kind="ExternalInput")
    out = nc.dram_tensor("out", [BPC, N, E], FP32, kind="ExternalOutput")

    from contextlib import ExitStack

    with tile.TileContext(nc) as tc:
        with ExitStack() as ctx:
            const = ctx.enter_context(tc.tile_pool(name="const", bufs=1))
            xtp = ctx.enter_context(tc.tile_pool(name="xtp", bufs=2 if fast else 1))
            vp = ctx.enter_context(tc.tile_pool(name="vp", bufs=2 if fast else 1))
            oallp = ctx.enter_context(tc.tile_pool(name="oallp", bufs=1))
            wqp = ctx.enter_context(tc.tile_pool(name="wqp", bufs=2))
            stgp = ctx.enter_context(tc.tile_pool(name="stgp", bufs=3))
            qtp = ctx.enter_context(tc.tile_pool(name="qtp", bufs=3))
            ktp = ctx.enter_context(tc.tile_pool(name="ktp", bufs=3))
            estp = ctx.enter_context(tc.tile_pool(name="estp", bufs=4 if fast else 3))
            rp = ctx.enter_context(tc.tile_pool(name="rp", bufs=4))
            rbcp = ctx.enter_context(tc.tile_pool(name="rbcp", bufs=2))
            obp = ctx.enter_context(tc.tile_pool(name="obp", bufs=2))
            qkps = ctx.enter_context(tc.tile_pool(name="qkps", bufs=2, space="PSUM"))
            stps = ctx.enter_context(tc.tile_pool(name="stps", bufs=2, space="PSUM"))
            ops = ctx.enter_context(tc.tile_pool(name="ops", bufs=2, space="PSUM"))
            # ---- resident constants (loads deferred into the batch loop) ----
            bqk_sb = const.tile([128, (H // 2) * 3], FP32)
            vb_sb = const.tile([128, H * VW], FP32)
            wv_sb = const.tile([128, EC * H * VW], DTF)
            pw_sb = const.tile([128, H * E], BF16)
            pb_sb = const.tile([128, E], FP32)

            for b in range(BPC):
                # first head-pair's weights issued ahead of x^T so the
                # first QK matmul has everything as early as possible
                wp0 = wqp.tile([128, EC * 4 * D], DTF, tag="wq", name=f"wp0_{b}")
                nc.sync.dma_start(wp0[:], wqk.ap()[0])

                # ---- x^T for this batch ----
                xt_sb = xtp.tile([128, EC * N], DTF, tag="xt")
                for c in range(EC):
                    nc.sync.dma_start(
                        xt_sb[:, c * N:(c + 1) * N], xt.ap()[b, :, c * N:(c + 1) * N]
                    )
                if b == 0:
                    nc.sync.dma_start(bqk_sb[:], bqk.ap())
                    nc.sync.dma_start(vb_sb[:], vb.ap())

                v_sb = vp.tile([128, TC * H * VW], DTF, tag="v")

                def emit_vgen():
                    for t in range(TC):
                        vg = stps.tile([128, 1024], FP32, tag="st",
                                       name=f"vg_{b}_{t}")
                        for lo, hi in ((0, 512), (512, H * VW)):
                            for c in range(EC):
                                nc.tensor.matmul(
                                    vg[:, lo:hi],
                                    xt_sb[:, c * N + t * 128: c * N + (t + 1) * 128],
                                    wv_sb[:, c * H * VW + lo: c * H * VW + hi],
                                    start=(c == 0),
                                    stop=(c == EC - 1),
                                )
                        nc.vector.tensor_tensor(
                            v_sb[:, t * H * VW:(t + 1) * H * VW],
                            vg[:, 0:H * VW], vb_sb[:], op=OP.add,
                        )

                # ---- O_all^T accumulator in padded-head layout ----
                o_all = oallp.tile([128, H * N], BF16, tag="oall")

                # piece table: (src_r0, src_r1, which, sub, dst_r0) per chunk
                PIECES = (
                    ((0, 96, "q", 0, 0), (96, 128, "k", 0, 0)),
                    ((0, 64, "k", 0, 32), (64, 128, "q", 1, 0)),
                    ((0, 32, "q", 1, 64), (32, 128, "k", 1, 0)),
                )
                qts = [None, None]
                kts = [None, None]
                for h in range(H):
                    p, sub = divmod(h, 2)
                    if sub == 0:
                        if p == 0:
                            wp = wp0
                        else:
                            wp = wqp.tile([128, EC * 4 * D], DTF, tag="wq")
                            nc.sync.dma_start(wp[:], wqk.ap()[p])
                        qts[0] = qtp.tile([D, N], DTF, tag="qt",
                                          name=f"qt_{b}_{h}")
                        qts[1] = qtp.tile([D, N], DTF, tag="qt",
                                          name=f"qt_{b}_{h}b")
                        kts[0] = ktp.tile([D, N], DTF, tag="kt",
                                          name=f"kt_{b}_{h}")
                        kts[1] = ktp.tile([D, N], DTF, tag="kt",
                                          name=f"kt_{b}_{h}b")
                        for m in range(3):
                            for qh in range(2):
                                g_ps = qkps.tile([128, 512], FP32, tag="qk")
                                for c in range(EC):
                                    nc.tensor.matmul(
                                        g_ps[:],
                                        wp[:, c * 4 * D + m * 128: c * 4 * D + (m + 1) * 128],
                                        xt_sb[:, c * N + qh * 512: c * N + (qh + 1) * 512],
                                        start=(c == 0),
                                        stop=(c == EC - 1),
                                    )
                                stg = stgp.tile([128, 512], DTF, tag="stg")
                                if qh == 0:
                                    nc.scalar.add(
                                        stg[:], g_ps[:],
                                        bqk_sb[:, p * 3 + m: p * 3 + m + 1],
                                    )
                                else:
                                    nc.vector.tensor_scalar_add(
                                        stg[:], g_ps[:],
                                        bqk_sb[:, p * 3 + m: p * 3 + m + 1],
                                    )
                                for r0, r1, which, psub, d0 in PIECES[m]:
                                    dstt = qts[psub] if which == "q" else kts[psub]
                                    nc.sync.dma_start(
                                        dstt[d0:d0 + (r1 - r0), qh * 512:(qh + 1) * 512],
                                        stg[r0:r1, :],
                                    )
                    qt = qts[sub]
                    kt = kts[sub]

                    if h == 0:
                        if b == 0:
                            for c in range(EC):
                                nc.sync.dma_start(
                                    wv_sb[:, c * H * VW:(c + 1) * H * VW],
                                    wv.ap()[:, c * H * VW:(c + 1) * H * VW],
                                )
                        emit_vgen()
                        if b == 0:
                            for hc in range(H):
                                nc.sync.dma_start(
                                    pw_sb[:, hc * E:(hc + 1) * E],
                                    pw.ap()[:, hc * E:(hc + 1) * E],
                                )
                            nc.sync.dma_start(pb_sb[:], pb.ap())

                    # S^T -> exp -> O, software pipelined over key chunks.
                    # Both q-halves share each weight load (same lhsT).
                    o_ps = [ops.tile([128, 512], FP32, tag="o", name=f"o_{b}_{h}_{i}") for i in range(2)]
                    ests = [None] * TC

                    def s_step(t):
                        st = stps.tile([128, 1024], FP32, tag="st")
                        for qh in range(2):
                            nc.tensor.matmul(
                                st[:, qh * 512:(qh + 1) * 512],
                                kt[:, t * 128:(t + 1) * 128],
                                qt[:, qh * 512:(qh + 1) * 512],
                                start=True,
                                stop=True,
                            )
                        est = estp.tile([128, 1024], DTF, tag="est")
                        nc.scalar.activation(est[:], st[:], AF.Exp)
                        ests[t] = est

                    def o_step(t):
                        for qh in range(2):
                            nc.tensor.matmul(
                                o_ps[qh][0:VW, :],
                                v_sb[:, t * H * VW + h * VW: t * H * VW + (h + 1) * VW],
                                ests[t][:, qh * 512:(qh + 1) * 512],
                                start=(t == 0),
                                stop=(t == TC - 1),
                            )

                    LAT = 1
                    for t in range(TC):
                        s_step(t)
                        if t >= LAT:
                            o_step(t - LAT)
                    for t in range(TC - LAT, TC):
                        o_step(t)

                    for qh in range(2):
                        r = rp.tile([1, 512], FP32, tag="r")
                        nc.vector.reciprocal_approx_fast(r[:], o_ps[qh][0:1, :])
                        rbc = rbcp.tile([VW, 512], FP32, tag="rbc")
                        nc.gpsimd.partition_broadcast(rbc[:], r[:])
                        nc.vector.scalar_tensor_tensor(
                            o_all[0:VW, h * N + qh * 512: h * N + (qh + 1) * 512],
                            o_ps[qh][0:VW, :],
                            SCALE,
                            rbc[:],
                            OP.mult,
                            OP.mult,
                        )

                # ---- output projection (psum split across st/o pools) ----
                for t in range(TC):
                    pja = stps.tile([128, 512], FP32, tag="st", name=f"pja_{b}_{t}")
                    pjb = ops.tile([128, 256], FP32, tag="o", name=f"pjb_{b}_{t}")
                    for hc in range(H):
                        lhsT = o_all[0:VW, hc * N + t * 128: hc * N + (t + 1) * 128]
                        nc.tensor.matmul(
                            pja[:], lhsT, pw_sb[0:VW, hc * E: hc * E + 512],
                            start=(hc == 0), stop=(hc == H - 1),
                        )
                        nc.tensor.matmul(
                            pjb[:], lhsT, pw_sb[0:VW, hc * E + 512: hc * E + E],
                            start=(hc == 0), stop=(hc == H - 1),
                        )
                    oba = obp.tile([128, 512], FP32, tag="oba")
                    nc.vector.tensor_tensor(oba[:], pja[:], pb_sb[:, 0:512], op=OP.add)
                    nc.sync.dma_start(out.ap()[b, t * 128:(t + 1) * 128, 0:512], oba[:])
                    obb = obp.tile([128, 256], FP32, tag="obb")
                    nc.vector.tensor_tensor(obb[:], pjb[:], pb_sb[:, 512:E], op=OP.add)
                    nc.sync.dma_start(out.ap()[b, t * 128:(t + 1) * 128, 512:E], obb[:])

    nc.compile()
    return nc


def get_nc():
    if "nc" not in _NC_CACHE:
        _NC_CACHE["nc"] = _build_nc()
    return _NC_CACHE["nc"]


def _prep_inputs(x, qkv_w, qkv_b, proj_w, proj_b):
    """Host-side layout prep shared by all cores + per-core x shards."""
    x = np.ascontiguousarray(x, dtype=np.float32)
    qkv_w = np.asarray(qkv_w, dtype=np.float32)
    qkv_b = np.asarray(qkv_b, dtype=np.float32)
    proj_w = np.asarray(proj_w, dtype=np.float32)
    proj_b = np.asarray(proj_b, dtype=np.float32)

    hh = np.arange(H)[:, None]
    dd = np.arange(D)[None, :]
    idx = [(hh * 3 * D + dd * 3 + c).reshape(-1) for c in range(3)]  # [768] each

    import ml_dtypes
    dtf = ml_dtypes.bfloat16 if PRECISION == "fast" else np.float32
    # packed head-pair QK weights: [H/2, 128, EC*4D]; per E-chunk the 384
    # feature cols are [Q_2p (96) | K_2p (96) | Q_2p+1 (96) | K_2p+1 (96)]
    wqT = qkv_w[idx[0], :].T.reshape(EC, 128, H, D)  # [c, p, h, d]
    wkT = qkv_w[idx[1], :].T.reshape(EC, 128, H, D)
    wqk_l = np.empty((H // 2, 128, EC, 4, D), dtype=np.float32)
    for pr in range(H // 2):
        wqk_l[pr, :, :, 0, :] = wqT[:, :, 2 * pr, :].transpose(1, 0, 2)
        wqk_l[pr, :, :, 1, :] = wkT[:, :, 2 * pr, :].transpose(1, 0, 2)
        wqk_l[pr, :, :, 2, :] = wqT[:, :, 2 * pr + 1, :].transpose(1, 0, 2)
        wqk_l[pr, :, :, 3, :] = wkT[:, :, 2 * pr + 1, :].transpose(1, 0, 2)
    wqk_l = np.ascontiguousarray(wqk_l.reshape(H // 2, 128, EC * 4 * D).astype(dtf))

    # wv: [128, EC*H*DP]; col c*H*DP + h*DP + d = qkv_w[idx2[h*D+d], c*128+p], pad 0
    wvT = qkv_w[idx[2], :].T.reshape(EC, 128, H, D)  # [c, p, h, d]
    wv_l = np.zeros((128, EC, H, VW), dtype=np.float32)
    wv_l[:, :, :, 1:D + 1] = wvT.transpose(1, 0, 2, 3)
    wv_l = np.ascontiguousarray(wv_l.reshape(128, EC * H * VW).astype(dtf))

    # vb: [128, H*DP] broadcast v-bias + ones column at d=D
    vb_row = np.zeros((H, VW), dtype=np.float32)
    vb_row[:, 1:D + 1] = qkv_b[idx[2]].reshape(H, D)
    vb_row[:, 0] = 1.0
    vb_l = np.ascontiguousarray(np.broadcast_to(vb_row.reshape(1, H * VW), (128, H * VW)))

    # bqk: [128, 3*H/2]; col p*3+m = per-partition bias for packed chunk m
    bq = qkv_b[idx[0]].reshape(H, D)
    bk = qkv_b[idx[1]].reshape(H, D)
    bqk_l = np.zeros((128, (H // 2) * 3), dtype=np.float32)
    for pr in range(H // 2):
        bqk_l[0:96, pr * 3 + 0] = bq[2 * pr]
        bqk_l[96:128, pr * 3 + 0] = bk[2 * pr][0:32]
        bqk_l[0:64, pr * 3 + 1] = bk[2 * pr][32:96]
        bqk_l[64:128, pr * 3 + 1] = bq[2 * pr + 1][0:64]
        bqk_l[0:32, pr * 3 + 2] = bq[2 * pr + 1][64:96]
        bqk_l[32:128, pr * 3 + 2] = bk[2 * pr + 1][0:96]

    # pw: [128, H*E]; pw_l[p, h*E+e] = proj_w[e, h*D+dd] for p=dd<D else 0
    pw_l = np.zeros((128, H, E), dtype=np.float32)
    pw_l[1:D + 1, :, :] = proj_w.reshape(E, H, D).transpose(2, 1, 0)
    pw_l = np.ascontiguousarray(pw_l.reshape(128, H * E).astype(ml_dtypes.bfloat16))

    pb_l = np.ascontiguousarray(np.broadcast_to(proj_b.reshape(1, E), (128, E)))

    # x^T per batch in sbuf layout: [B, 128, EC*N]; [b, p, c*N+n] = x[b, n, c*128+p]
    xt_all = np.ascontiguousarray(
        x.reshape(B, N, EC, 128).transpose(0, 3, 2, 1).reshape(B, 128, EC * N)
    ).astype(dtf)

    in_maps = []
    for core in range(NCORES):
        xt_core = np.ascontiguousarray(
            xt_all[core * BPC:(core + 1) * BPC]
        )
        in_maps.append(
            {
                "xt": xt_core,
                "wqk": wqk_l,
                "wv": wv_l,
                "vb": vb_l,
                "bqk": bqk_l,
                "pw": pw_l,
                "pb": pb_l,
            }
        )
    return in_maps


def run(inputs, trace=False):
    from concourse.bass_utils import run_bass_kernel_spmd

    nc = get_nc()
    in_maps = _prep_inputs(**inputs)
    res = run_bass_kernel_spmd(
        nc, in_maps, core_ids=list(range(NCORES)), trace=trace
    )
    out = np.concatenate([res.results[c]["out"] for c in range(NCORES)], axis=0)
    return out, res


def kernel(**inputs) -> np.ndarray:
    out, _ = run(inputs, trace=False)
    return out


# revision 22
# speedup vs baseline: 1.1766x; 1.1766x over previous
"""Multi-head attention (B=16, N=1024, E=768, H=8) on 8 Trainium2 NeuronCores.

Sharding: data-parallel over batch (2 batches per core, no collectives).
Per core, one fused Tile kernel:
  - host pre-transposes x -> x^T and pre-permutes the interleaved qkv weights
    (including packing head-pair Q|K features into full 128-row chunks so the
    QK projection runs at 100% PE utilization; pieces are unscrambled into
    per-head Q^T/K^T tiles via staged copies + partition-shift SBUF DMAs)
  - V is produced per batch for all heads, 97 cols per head: a leading ones
    column (so the softmax denominator falls out of the O matmul as row 0)
    plus the 96 V columns
  - S^T = (K^T)^T @ Q^T -> PSUM, Exp on the scalar engine -> SBUF
  - O = V''^T @ exp(S^T) accumulated over key chunks, software-pipelined
    with the S matmuls (each weight load shared by both q-halves)
  - normalize with reciprocal_approx_fast + gpsimd partition_broadcast + one
    fused DVE multiply that also folds the post-softmax 1/sqrt(E) scale
  - output projection from the transposed O layout (contraction sliced to 97
    rows); bias added on DVE, DMA out
Matmuls run in bf16 (PRECISION="fast") or fp32r (="safe", ~11%% slower,
~6x lower error); softmax/accumulation stays fp32.
"""
import sys
import os

for _p in ("/opt/trn_rl_repo", "/root/.axon_site", "/root/.axon_site/_ro/trn_rl_repo"):
    if os.path.isdir(_p) and _p not in sys.path:
        sys.path.append(_p)

import numpy as np

B, N, E, H = 16, 1024, 768, 8
D = E // H            # 96
NCORES = 8
BPC = B // NCORES     # batches per core = 2
EC = E // 128         # 6 E-chunks
TC = N // 128         # 8 token chunks
DP = 128              # padded per-head width in the proj layout
VW = D + 1            # per-head width in the V layout (ones col + 96 V cols)
SCALE = float(1.0 / np.sqrt(np.float32(E)))

# "fast": bf16 activations/weights on the attention path (~0.6% scale absmax)
# "safe": fp32r (tf32-like) everywhere (~0.04% scale absmax), ~8% slower
PRECISION = "fast"

_NC_CACHE = {}


def _build_nc():
    import concourse.bacc as bacc
    import concourse.mybir as mybir
    import concourse.tile as tile

    FP32 = mybir.dt.float32
    FP32R = mybir.dt.float32r
    BF16 = mybir.dt.bfloat16
    DTF = BF16 if PRECISION == "fast" else FP32R
    AF = mybir.ActivationFunctionType
    OP = mybir.AluOpType

    fast = PRECISION == "fast"
    nc = bacc.Bacc("TRN2", target_bir_lowering=False, debug=False, num_devices=NCORES)

    xt = nc.dram_tensor("xt", [BPC, 128, EC * N], DTF, kind="ExternalInput")
    wqk = nc.dram_tensor("wqk", [H // 2, 128, EC * 4 * D], DTF, kind="ExternalInput")
    wv = nc.dram_tensor("wv", [128, EC * H * VW], DTF, kind="ExternalInput")
    vb = nc.dram_tensor("vb", [128, H * VW], FP32, kind="ExternalInput")
    bqk = nc.dram_tensor("bqk", [128, (H // 2) * 3], FP32, kind="ExternalInput")
    pw = nc.dram_tensor("pw", [128, H * E], BF16, kind="ExternalInput")
    pb = nc.dram_tensor("pb", [128, E], FP32, kind="ExternalInput")
    out = nc.dram_tensor("out", [BPC, N, E], FP32, kind="ExternalOutput")

    from contextlib import ExitStack

    with tile.TileContext(nc) as tc:
        with ExitStack() as ctx:
            const = ctx.enter_context(tc.tile_pool(name="const", bufs=1))
            xtp = ctx.enter_context(tc.tile_pool(name="xtp", bufs=2 if fast else 1))
            vp = ctx.enter_context(tc.tile_pool(name="vp", bufs=2 if fast else 1))
            oallp = ctx.enter_context(tc.tile_pool(name="oallp", bufs=1))
            wqp = ctx.enter_context(tc.tile_pool(name="wqp", bufs=2))
            stgp = ctx.enter_context(tc.tile_pool(name="stgp", bufs=4))
            qtp = ctx.enter_context(tc.tile_pool(name="qtp", bufs=5))
            ktp = ctx.enter_context(tc.tile_pool(name="ktp", bufs=5))
            estp = ctx.enter_context(tc.tile_pool(name="estp", bufs=4 if fast else 3))
            rp = ctx.enter_context(tc.tile_pool(name="rp", bufs=4))
            rbcp = ctx.enter_context(tc.tile_pool(name="rbcp", bufs=2))
            obp = ctx.enter_context(tc.tile_pool(name="obp", bufs=2))
            qkps = ctx.enter_context(tc.tile_pool(name="qkps", bufs=2, space="PSUM"))
            stps = ctx.enter_context(tc.tile_pool(name="stps", bufs=2, space="PSUM"))
            ops = ctx.enter_context(tc.tile_pool(name="ops", bufs=2, space="PSUM"))
            # ---- resident constants (loads deferred into the batch loop) ----
            bqk_sb = const.tile([128, (H // 2) * 3], FP32)
            vb_sb = const.tile([128, H * VW], FP32)
            wv_sb = const.tile([128, EC * H * VW], DTF)
            pw_sb = const.tile([128, H * E], BF16)
            pb_sb = const.tile([128, E], FP32)

            for b in range(BPC):
                # first head-pair's weights issued ahead of x^T so the
                # first QK matmul has everything as early as possible
                wp0 = wqp.tile([128, EC * 4 * D], DTF, tag="wq", name=f"wp0_{b}")
                nc.sync.dma_start(wp0[:], wqk.ap()[0])

                # ---- x^T for this batch ----
                xt_sb = xtp.tile([128, EC * N], DTF, tag="xt")
                for c in range(EC):
                    nc.sync.dma_start(
                        xt_sb[:, c * N:(c + 1) * N], xt.ap()[b, :, c * N:(c + 1) * N]
                    )
                if b == 0:
                    nc.sync.dma_start(bqk_sb[:], bqk.ap())
                    nc.sync.dma_start(vb_sb[:], vb.ap())

                v_sb = vp.tile([128, TC * H * VW], DTF, tag="v")

                def emit_vgen():
                    for t in range(TC):
                        vg = stps.tile([128, 1024], FP32, tag="st",
                                       name=f"vg_{b}_{t}")
                        for lo, hi in ((0, 512), (512, H * VW)):
                            for c in range(EC):
                                nc.tensor.matmul(
                                    vg[:, lo:hi],
                                    xt_sb[:, c * N + t * 128: c * N + (t + 1) * 128],
                                    wv_sb[:, c * H * VW + lo: c * H * VW + hi],
                                    start=(c == 0),
                                    stop=(c == EC - 1),
                                )
                        nc.vector.tensor_tensor(
                            v_sb[:, t * H * VW:(t + 1) * H * VW],
                            vg[:, 0:H * VW], vb_sb[:], op=OP.add,
                        )

                # ---- O_all^T accumulator in padded-head layout ----
                o_all = oallp.tile([128, H * N], BF16, tag="oall")

                # piece table: (src_r0, src_r1, which, sub, dst_r0) per chunk
                PIECES = (
                    ((0, 96, "q", 0, 0), (96, 128, "k", 0, 0)),
                    ((0, 64, "k", 0, 32), (64, 128, "q", 1, 0)),
                    ((0, 32, "q", 1, 64), (32, 128, "k", 1, 0)),
                )
                qt_by_head = {}
                kt_by_head = {}

                def emit_pair_gen(p):
                    wp = wp0 if p == 0 else wqp.tile(
                        [128, EC * 4 * D], DTF, tag="wq", name=f"wp_{b}_{p}"
                    )
                    if p > 0:
                        nc.sync.dma_start(wp[:], wqk.ap()[p])
                    pq = [
                        qtp.tile([D, N], DTF, tag="qt", name=f"qt_{b}_{2 * p + i}")
                        for i in range(2)
                    ]
                    pk = [
                        ktp.tile([D, N], DTF, tag="kt", name=f"kt_{b}_{2 * p + i}")
                        for i in range(2)
                    ]
                    qt_by_head[2 * p] = pq[0]
                    qt_by_head[2 * p + 1] = pq[1]
                    kt_by_head[2 * p] = pk[0]
                    kt_by_head[2 * p + 1] = pk[1]
                    for m in range(3):
                        for qh in range(2):
                            g_ps = qkps.tile([128, 512], FP32, tag="qk",
                                             name=f"g_{b}_{p}_{m}_{qh}")
                            for c in range(EC):
                                nc.tensor.matmul(
                                    g_ps[:],
                                    wp[:, c * 4 * D + m * 128: c * 4 * D + (m + 1) * 128],
                                    xt_sb[:, c * N + qh * 512: c * N + (qh + 1) * 512],
                                    start=(c == 0),
                                    stop=(c == EC - 1),
                                )
                            stg = stgp.tile([128, 512], DTF, tag="stg",
                                            name=f"stg_{b}_{p}_{m}_{qh}")
                            if qh == 0:
                                nc.scalar.add(
                                    stg[:], g_ps[:],
                                    bqk_sb[:, p * 3 + m: p * 3 + m + 1],
                                )
                            else:
                                nc.vector.tensor_scalar_add(
                                    stg[:], g_ps[:],
                                    bqk_sb[:, p * 3 + m: p * 3 + m + 1],
                                )
                            for r0, r1, which, psub, d0 in PIECES[m]:
                                dstt = (pq if which == "q" else pk)[psub]
                                nc.sync.dma_start(
                                    dstt[d0:d0 + (r1 - r0), qh * 512:(qh + 1) * 512],
                                    stg[r0:r1, :],
                                )

                for h in range(H):
                    if h not in qt_by_head:
                        emit_pair_gen(h // 2)
                    qt = qt_by_head[h]
                    kt = kt_by_head[h]

                    if h == 0:
                        if b == 0:
                            for c in range(EC):
                                nc.sync.dma_start(
                                    wv_sb[:, c * H * VW:(c + 1) * H * VW],
                                    wv.ap()[:, c * H * VW:(c + 1) * H * VW],
                                )
                        emit_pair_gen(1)
                        emit_vgen()
                        if b == 0:
                            for hc in range(H):
                                nc.sync.dma_start(
                                    pw_sb[:, hc * E:(hc + 1) * E],
                                    pw.ap()[:, hc * E:(hc + 1) * E],
                                )
                            nc.sync.dma_start(pb_sb[:], pb.ap())

                    # S^T -> exp -> O, software pipelined over key chunks.
                    # Both q-halves share each weight load (same lhsT).
                    o_ps = [ops.tile([128, 512], FP32, tag="o", name=f"o_{b}_{h}_{i}") for i in range(2)]
                    ests = [None] * TC

                    def s_step(t):
                        st = stps.tile([128, 1024], FP32, tag="st")
                        for qh in range(2):
                            nc.tensor.matmul(
                                st[:, qh * 512:(qh + 1) * 512],
                                kt[:, t * 128:(t + 1) * 128],
                                qt[:, qh * 512:(qh + 1) * 512],
                                start=True,
                                stop=True,
                            )
                        est = estp.tile([128, 1024], DTF, tag="est")
                        nc.scalar.activation(est[:], st[:], AF.Exp)
                        ests[t] = est

                    def o_step(t):
                        for qh in range(2):
                            nc.tensor.matmul(
                                o_ps[qh][0:VW, :],
                                v_sb[:, t * H * VW + h * VW: t * H * VW + (h + 1) * VW],
                                ests[t][:, qh * 512:(qh + 1) * 512],
                                start=(t == 0),
                                stop=(t == TC - 1),
                            )

                    LAT = 1
                    for t in range(TC):
                        s_step(t)
                        if t >= LAT:
                            o_step(t - LAT)
                    for t in range(TC - LAT, TC):
                        o_step(t)

                    for qh in range(2):
                        r = rp.tile([1, 512], FP32, tag="r")
                        nc.vector.reciprocal_approx_fast(r[:], o_ps[qh][0:1, :])
                        rbc = rbcp.tile([VW, 512], FP32, tag="rbc")
                        nc.gpsimd.partition_broadcast(rbc[:], r[:])
                        nc.vector.scalar_tensor_tensor(
                            o_all[0:VW, h * N + qh * 512: h * N + (qh + 1) * 512],
                            o_ps[qh][0:VW, :],
                            SCALE,
                            rbc[:],
                            OP.mult,
                            OP.mult,
                        )

                # ---- output projection (psum split across st/o pools) ----
                for t in range(TC):
                    pja = stps.tile([128, 512], FP32, tag="st", name=f"pja_{b}_{t}")
                    pjb = ops.tile([128, 256], FP32, tag="o", name=f"pjb_{b}_{t}")
                    for hc in range(H):
                        lhsT = o_all[0:VW, hc * N + t * 128: hc * N + (t + 1) * 128]
                        nc.tensor.matmul(
                            pja[:], lhsT, pw_sb[0:VW, hc * E: hc * E + 512],
                            start=(hc == 0), stop=(hc == H - 1),
                        )
                        nc.tensor.matmul(
                            pjb[:], lhsT, pw_sb[0:VW, hc * E + 512: hc * E + E],
                            start=(hc == 0), stop=(hc == H - 1),
                        )
                    oba = obp.tile([128, 512], FP32, tag="oba")
                    nc.vector.tensor_tensor(oba[:], pja[:], pb_sb[:, 0:512], op=OP.add)
                    nc.sync.dma_start(out.ap()[b, t * 128:(t + 1) * 128, 0:512], oba[:])
                    obb = obp.tile([128, 256], FP32, tag="obb")
                    nc.vector.tensor_tensor(obb[:], pjb[:], pb_sb[:, 512:E], op=OP.add)
                    nc.sync.dma_start(out.ap()[b, t * 128:(t + 1) * 128, 512:E], obb[:])

    nc.compile()
    return nc


def get_nc():
    if "nc" not in _NC_CACHE:
        _NC_CACHE["nc"] = _build_nc()
    return _NC_CACHE["nc"]


def _prep_inputs(x, qkv_w, qkv_b, proj_w, proj_b):
    """Host-side layout prep shared by all cores + per-core x shards."""
    x = np.ascontiguousarray(x, dtype=np.float32)
    qkv_w = np.asarray(qkv_w, dtype=np.float32)
    qkv_b = np.asarray(qkv_b, dtype=np.float32)
    proj_w = np.asarray(proj_w, dtype=np.float32)
    proj_b = np.asarray(proj_b, dtype=np.float32)

    hh = np.arange(H)[:, None]
    dd = np.arange(D)[None, :]
    idx = [(hh * 3 * D + dd * 3 + c).reshape(-1) for c in range(3)]  # [768] each

    import ml_dtypes
    dtf = ml_dtypes.bfloat16 if PRECISION == "fast" else np.float32
    # packed head-pair QK weights: [H/2, 128, EC*4D]; per E-chunk the 384
    # feature cols are [Q_2p (96) | K_2p (96) | Q_2p+1 (96) | K_2p+1 (96)]
    wqT = qkv_w[idx[0], :].T.reshape(EC, 128, H, D)  # [c, p, h, d]
    wkT = qkv_w[idx[1], :].T.reshape(EC, 128, H, D)
    wqk_l = np.empty((H // 2, 128, EC, 4, D), dtype=np.float32)
    for pr in range(H // 2):
        wqk_l[pr, :, :, 0, :] = wqT[:, :, 2 * pr, :].transpose(1, 0, 2)
        wqk_l[pr, :, :, 1, :] = wkT[:, :, 2 * pr, :].transpose(1, 0, 2)
        wqk_l[pr, :, :, 2, :] = wqT[:, :, 2 * pr + 1, :].transpose(1, 0, 2)
        wqk_l[pr, :, :, 3, :] = wkT[:, :, 2 * pr + 1, :].transpose(1, 0, 2)
    wqk_l = np.ascontiguousarray(wqk_l.reshape(H // 2, 128, EC * 4 * D).astype(dtf))

    # wv: [128, EC*H*DP]; col c*H*DP + h*DP + d = qkv_w[idx2[h*D+d], c*128+p], pad 0
    wvT = qkv_w[idx[2], :].T.reshape(EC, 128, H, D)  # [c, p, h, d]
    wv_l = np.zeros((128, EC, H, VW), dtype=np.float32)
    wv_l[:, :, :, 1:D + 1] = wvT.transpose(1, 0, 2, 3)
    wv_l = np.ascontiguousarray(wv_l.reshape(128, EC * H * VW).astype(dtf))

    # vb: [128, H*DP] broadcast v-bias + ones column at d=D
    vb_row = np.zeros((H, VW), dtype=np.float32)
    vb_row[:, 1:D + 1] = qkv_b[idx[2]].reshape(H, D)
    vb_row[:, 0] = 1.0
    vb_l = np.ascontiguousarray(np.broadcast_to(vb_row.reshape(1, H * VW), (128, H * VW)))

    # bqk: [128, 3*H/2]; col p*3+m = per-partition bias for packed chunk m
    bq = qkv_b[idx[0]].reshape(H, D)
    bk = qkv_b[idx[1]].reshape(H, D)
    bqk_l = np.zeros((128, (H // 2) * 3), dtype=np.float32)
    for pr in range(H // 2):
        bqk_l[0:96, pr * 3 + 0] = bq[2 * pr]
        bqk_l[96:128, pr * 3 + 0] = bk[2 * pr][0:32]
        bqk_l[0:64, pr * 3 + 1] = bk[2 * pr][32:96]
        bqk_l[64:128, pr * 3 + 1] = bq[2 * pr + 1][0:64]
        bqk_l[0:32, pr * 3 + 2] = bq[2 * pr + 1][64:96]
        bqk_l[32:128, pr * 3 + 2] = bk[2 * pr + 1][0:96]

    # pw: [128, H*E]; pw_l[p, h*E+e] = proj_w[e, h*D+dd] for p=dd<D else 0
    pw_l = np.zeros((128, H, E), dtype=np.float32)
    pw_l[1:D + 1, :, :] = proj_w.reshape(E, H, D).transpose(2, 1, 0)
    pw_l = np.ascontiguousarray(pw_l.reshape(128, H * E).astype(ml_dtypes.bfloat16))

    pb_l = np.ascontiguousarray(np.broadcast_to(proj_b.reshape(1, E), (128, E)))

    # x^T per batch in sbuf layout: [B, 128, EC*N]; [b, p, c*N+n] = x[b, n, c*128+p]
    xt_all = np.ascontiguousarray(
        x.reshape(B, N, EC, 128).transpose(0, 3, 2, 1).reshape(B, 128, EC * N)
    ).astype(dtf)

    in_maps = []
    for core in range(NCORES):
        xt_core = np.ascontiguousarray(
            xt_all[core * BPC:(core + 1) * BPC]
        )
        in_maps.append(
            {
                "xt": xt_core,
                "wqk": wqk_l,
                "wv": wv_l,
                "vb": vb_l,
                "bqk": bqk_l,
                "pw": pw_l,
                "pb": pb_l,
            }
        )
    return in_maps


def run(inputs, trace=False):
    from concourse.bass_utils import run_bass_kernel_spmd

    nc = get_nc()
    in_maps = _prep_inputs(**inputs)
    res = run_bass_kernel_spmd(
        nc, in_maps, core_ids=list(range(NCORES)), trace=trace
    )
    out = np.concatenate([res.results[c]["out"] for c in range(NCORES)], axis=0)
    return out, res


def kernel(**inputs) -> np.ndarray:
    out, _ = run(inputs, trace=False)
    return out


# revision 23
# speedup vs baseline: 1.1959x; 1.0164x over previous
"""Multi-head attention (B=16, N=1024, E=768, H=8) on 8 Trainium2 NeuronCores.

Sharding: data-parallel over batch (2 batches per core, no collectives).
Per core, one fused Tile kernel:
  - host pre-transposes x -> x^T and pre-permutes the interleaved qkv weights
    (including packing head-pair Q|K features into full 128-row chunks so the
    QK projection runs at 100% PE utilization; pieces are unscrambled into
    per-head Q^T/K^T tiles via staged copies + partition-shift SBUF DMAs)
  - V is produced per batch for all heads, 97 cols per head: a leading ones
    column (so the softmax denominator falls out of the O matmul as row 0)
    plus the 96 V columns
  - S^T = (K^T)^T @ Q^T -> PSUM, Exp on the scalar engine -> SBUF
  - O = V''^T @ exp(S^T) accumulated over key chunks, software-pipelined
    with the S matmuls (each weight load shared by both q-halves)
  - normalize with reciprocal_approx_fast + gpsimd partition_broadcast + one
    fused DVE multiply that also folds the post-softmax 1/sqrt(E) scale
  - output projection from the transposed O layout (contraction sliced to 97
    rows); bias added on DVE, DMA out
Matmuls run in bf16 (PRECISION="fast") or fp32r (="safe", ~11%% slower,
~6x lower error); softmax/accumulation stays fp32.
"""
import sys
import os

for _p in ("/opt/trn_rl_repo", "/root/.axon_site", "/root/.axon_site/_ro/trn_rl_repo"):
    if os.path.isdir(_p) and _p not in sys.path:
        sys.path.append(_p)

import numpy as np

B, N, E, H = 16, 1024, 768, 8
D = E // H            # 96
NCORES = 8
BPC = B // NCORES     # batches per core = 2
EC = E // 128         # 6 E-chunks
TC = N // 128         # 8 token chunks
DP = 128              # padded per-head width in the proj layout
VW = D + 1            # per-head width in the V layout (ones col + 96 V cols)
SCALE = float(1.0 / np.sqrt(np.float32(E)))

# "fast": bf16 activations/weights on the attention path (~0.6% scale absmax)
# "safe": fp32r (tf32-like) everywhere (~0.04% scale absmax), ~8% slower
PRECISION = "fast"

_NC_CACHE = {}


def _build_nc():
    import concourse.bacc as bacc
    import concourse.mybir as mybir
    import concourse.tile as tile

    FP32 = mybir.dt.float32
    FP32R = mybir.dt.float32r
    BF16 = mybir.dt.bfloat16
    DTF = BF16 if PRECISION == "fast" else FP32R
    AF = mybir.ActivationFunctionType
    OP = mybir.AluOpType

    fast = PRECISION == "fast"
    nc = bacc.Bacc("TRN2", target_bir_lowering=False, debug=False, num_devices=NCORES)

    xt = nc.dram_tensor("xt", [BPC, 128, EC * N], DTF, kind="ExternalInput")
    wqk = nc.dram_tensor("wqk", [H // 2, 128, EC * 4 * D], DTF, kind="ExternalInput")
    wv = nc.dram_tensor("wv", [128, EC * H * VW], DTF, kind="ExternalInput")
    vb = nc.dram_tensor("vb", [128, H * VW], FP32, kind="ExternalInput")
    bqk = nc.dram_tensor("bqk", [128, (H // 2) * 3], FP32, kind="ExternalInput")
    pw = nc.dram_tensor("pw", [128, H * E], BF16, kind="ExternalInput")
    pb = nc.dram_tensor("pb", [128, E], FP32, kind="ExternalInput")
    out = nc.dram_tensor("out", [BPC, N, E], FP32, kind="ExternalOutput")

    from contextlib import ExitStack

    with tile.TileContext(nc) as tc:
        with ExitStack() as ctx:
            const = ctx.enter_context(tc.tile_pool(name="const", bufs=1))
            xtp = ctx.enter_context(tc.tile_pool(name="xtp", bufs=2 if fast else 1))
            vp = ctx.enter_context(tc.tile_pool(name="vp", bufs=2 if fast else 1))
            oallp = ctx.enter_context(tc.tile_pool(name="oallp", bufs=1))
            wqp = ctx.enter_context(tc.tile_pool(name="wqp", bufs=2))
            stgp = ctx.enter_context(tc.tile_pool(name="stgp", bufs=3))
            qtp = ctx.enter_context(tc.tile_pool(name="qtp", bufs=3))
            ktp = ctx.enter_context(tc.tile_pool(name="ktp", bufs=3))
            estp = ctx.enter_context(tc.tile_pool(name="estp", bufs=4 if fast else 3))
            rp = ctx.enter_context(tc.tile_pool(name="rp", bufs=4))
            rbcp = ctx.enter_context(tc.tile_pool(name="rbcp", bufs=2))
            obp = ctx.enter_context(tc.tile_pool(name="obp", bufs=2))
            qkps = ctx.enter_context(tc.tile_pool(name="qkps", bufs=2, space="PSUM"))
            stps = ctx.enter_context(tc.tile_pool(name="stps", bufs=2, space="PSUM"))
            ops = ctx.enter_context(tc.tile_pool(name="ops", bufs=2, space="PSUM"))
            # ---- resident constants (loads deferred into the batch loop) ----
            bqk_sb = const.tile([128, (H // 2) * 3], FP32)
            vb_sb = const.tile([128, H * VW], FP32)
            wv_sb = const.tile([128, EC * H * VW], DTF)
            pw_sb = const.tile([128, H * E], BF16)
            pb_sb = const.tile([128, E], FP32)

            for b in range(BPC):
                # first head-pair's weights issued ahead of x^T so the
                # first QK matmul has everything as early as possible
                wp0 = wqp.tile([128, EC * 4 * D], DTF, tag="wq", name=f"wp0_{b}")
                nc.sync.dma_start(wp0[:], wqk.ap()[0])

                # ---- x^T for this batch ----
                xt_sb = xtp.tile([128, EC * N], DTF, tag="xt")
                for c in range(EC):
                    nc.sync.dma_start(
                        xt_sb[:, c * N:(c + 1) * N], xt.ap()[b, :, c * N:(c + 1) * N]
                    )
                if b == 0:
                    nc.sync.dma_start(bqk_sb[:], bqk.ap())
                    nc.sync.dma_start(vb_sb[:], vb.ap())

                v_sb = vp.tile([128, TC * H * VW], DTF, tag="v")

                def emit_vgen():
                    for t in range(TC):
                        vg = stps.tile([128, 1024], FP32, tag="st",
                                       name=f"vg_{b}_{t}")
                        for lo, hi in ((0, 512), (512, H * VW)):
                            for c in range(EC):
                                nc.tensor.matmul(
                                    vg[:, lo:hi],
                                    xt_sb[:, c * N + t * 128: c * N + (t + 1) * 128],
                                    wv_sb[:, c * H * VW + lo: c * H * VW + hi],
                                    start=(c == 0),
                                    stop=(c == EC - 1),
                                )
                        nc.vector.tensor_tensor(
                            v_sb[:, t * H * VW:(t + 1) * H * VW],
                            vg[:, 0:H * VW], vb_sb[:], op=OP.add,
                        )

                # ---- O_all^T accumulator in padded-head layout ----
                o_all = oallp.tile([128, H * N], BF16, tag="oall")

                # piece table: (src_r0, src_r1, which, sub, dst_r0) per chunk
                PIECES = (
                    ((0, 96, "q", 0, 0), (96, 128, "k", 0, 0)),
                    ((0, 64, "k", 0, 32), (64, 128, "q", 1, 0)),
                    ((0, 32, "q", 1, 64), (32, 128, "k", 1, 0)),
                )
                qt_by_head = {}
                kt_by_head = {}

                def emit_pair_gen(p):
                    wp = wp0 if p == 0 else wqp.tile(
                        [128, EC * 4 * D], DTF, tag="wq", name=f"wp_{b}_{p}"
                    )
                    if p > 0:
                        nc.sync.dma_start(wp[:], wqk.ap()[p])
                    pq = [
                        qtp.tile([D, N], DTF, tag="qt", name=f"qt_{b}_{2 * p + i}")
                        for i in range(2)
                    ]
                    pk = [
                        ktp.tile([D, N], DTF, tag="kt", name=f"kt_{b}_{2 * p + i}")
                        for i in range(2)
                    ]
                    qt_by_head[2 * p] = pq[0]
                    qt_by_head[2 * p + 1] = pq[1]
                    kt_by_head[2 * p] = pk[0]
                    kt_by_head[2 * p + 1] = pk[1]
                    for m in range(3):
                        for qh in range(2):
                            g_ps = qkps.tile([128, 512], FP32, tag="qk",
                                             name=f"g_{b}_{p}_{m}_{qh}")
                            for c in range(EC):
                                nc.tensor.matmul(
                                    g_ps[:],
                                    wp[:, c * 4 * D + m * 128: c * 4 * D + (m + 1) * 128],
                                    xt_sb[:, c * N + qh * 512: c * N + (qh + 1) * 512],
                                    start=(c == 0),
                                    stop=(c == EC - 1),
                                )
                            stg = stgp.tile([128, 512], DTF, tag="stg",
                                            name=f"stg_{b}_{p}_{m}_{qh}")
                            if qh == 0:
                                nc.scalar.add(
                                    stg[:], g_ps[:],
                                    bqk_sb[:, p * 3 + m: p * 3 + m + 1],
                                )
                            else:
                                nc.vector.tensor_scalar_add(
                                    stg[:], g_ps[:],
                                    bqk_sb[:, p * 3 + m: p * 3 + m + 1],
                                )
                            for r0, r1, which, psub, d0 in PIECES[m]:
                                dstt = (pq if which == "q" else pk)[psub]
                                nc.sync.dma_start(
                                    dstt[d0:d0 + (r1 - r0), qh * 512:(qh + 1) * 512],
                                    stg[r0:r1, :],
                                )

                for h in range(H):
                    if h not in qt_by_head:
                        emit_pair_gen(h // 2)
                    qt = qt_by_head[h]
                    kt = kt_by_head[h]

                    if h == 0:
                        if b == 0:
                            for c in range(EC):
                                nc.sync.dma_start(
                                    wv_sb[:, c * H * VW:(c + 1) * H * VW],
                                    wv.ap()[:, c * H * VW:(c + 1) * H * VW],
                                )
                        emit_vgen()
                        if b == 0:
                            for hc in range(H):
                                nc.sync.dma_start(
                                    pw_sb[:, hc * E:(hc + 1) * E],
                                    pw.ap()[:, hc * E:(hc + 1) * E],
                                )
                            nc.sync.dma_start(pb_sb[:], pb.ap())

                    # S^T -> exp -> O, software pipelined over key chunks.
                    # Both q-halves share each weight load (same lhsT).
                    o_ps = [ops.tile([128, 512], FP32, tag="o", name=f"o_{b}_{h}_{i}") for i in range(2)]
                    ests = [None] * TC

                    def s_step(t):
                        st = stps.tile([128, 1024], FP32, tag="st")
                        for qh in range(2):
                            nc.tensor.matmul(
                                st[:, qh * 512:(qh + 1) * 512],
                                kt[:, t * 128:(t + 1) * 128],
                                qt[:, qh * 512:(qh + 1) * 512],
                                start=True,
                                stop=True,
                            )
                        est = estp.tile([128, 1024], DTF, tag="est")
                        nc.scalar.activation(est[:], st[:], AF.Exp)
                        ests[t] = est

                    def o_step(t):
                        for qh in range(2):
                            nc.tensor.matmul(
                                o_ps[qh][0:VW, :],
                                v_sb[:, t * H * VW + h * VW: t * H * VW + (h + 1) * VW],
                                ests[t][:, qh * 512:(qh + 1) * 512],
                                start=(t == 0),
                                stop=(t == TC - 1),
                            )

                    LAT = 1
                    for t in range(TC):
                        s_step(t)
                        if t >= LAT:
                            o_step(t - LAT)
                    for t in range(TC - LAT, TC):
                        o_step(t)

                    for qh in range(2):
                        r = rp.tile([1, 512], FP32, tag="r")
                        nc.vector.reciprocal_approx_fast(r[:], o_ps[qh][0:1, :])
                        rbc = rbcp.tile([VW, 512], FP32, tag="rbc")
                        nc.gpsimd.partition_broadcast(rbc[:], r[:])
                        nc.vector.scalar_tensor_tensor(
                            o_all[0:VW, h * N + qh * 512: h * N + (qh + 1) * 512],
                            o_ps[qh][0:VW, :],
                            SCALE,
                            rbc[:],
                            OP.mult,
                            OP.mult,
                        )

                # ---- output projection (psum split across st/o pools) ----
                for t in range(TC):
                    pja = stps.tile([128, 512], FP32, tag="st", name=f"pja_{b}_{t}")
                    pjb = ops.tile([128, 256], FP32, tag="o", name=f"pjb_{b}_{t}")
                    for hc in range(H):
                        lhsT = o_all[0:VW, hc * N + t * 128: hc * N + (t + 1) * 128]
                        nc.tensor.matmul(
                            pja[:], lhsT, pw_sb[0:VW, hc * E: hc * E + 512],
                            start=(hc == 0), stop=(hc == H - 1),
                        )
                        nc.tensor.matmul(
                            pjb[:], lhsT, pw_sb[0:VW, hc * E + 512: hc * E + E],
                            start=(hc == 0), stop=(hc == H - 1),
                        )
                    oba = obp.tile([128, 512], FP32, tag="oba")
                    nc.vector.tensor_tensor(oba[:], pja[:], pb_sb[:, 0:512], op=OP.add)
                    nc.sync.dma_start(out.ap()[b, t * 128:(t + 1) * 128, 0:512], oba[:])
                    obb = obp.tile([128, 256], FP32, tag="obb")
                    nc.vector.tensor_tensor(obb[:], pjb[:], pb_sb[:, 512:E], op=OP.add)
                    nc.sync.dma_start(out.ap()[b, t * 128:(t + 1) * 128, 512:E], obb[:])

    nc.compile()
    return nc


def get_nc():
    if "nc" not in _NC_CACHE:
        _NC_CACHE["nc"] = _build_nc()
    return _NC_CACHE["nc"]


def _prep_inputs(x, qkv_w, qkv_b, proj_w, proj_b):
    """Host-side layout prep shared by all cores + per-core x shards."""
    x = np.ascontiguousarray(x, dtype=np.float32)
    qkv_w = np.asarray(qkv_w, dtype=np.float32)
    qkv_b = np.asarray(qkv_b, dtype=np.float32)
    proj_w = np.asarray(proj_w, dtype=np.float32)
    proj_b = np.asarray(proj_b, dtype=np.float32)

    hh = np.arange(H)[:, None]
    dd = np.arange(D)[None, :]
    idx = [(hh * 3 * D + dd * 3 + c).reshape(-1) for c in range(3)]  # [768] each

    import ml_dtypes
    dtf = ml_dtypes.bfloat16 if PRECISION == "fast" else np.float32
    # packed head-pair QK weights: [H/2, 128, EC*4D]; per E-chunk the 384
    # feature cols are [Q_2p (96) | K_2p (96) | Q_2p+1 (96) | K_2p+1 (96)]
    wqT = qkv_w[idx[0], :].T.reshape(EC, 128, H, D)  # [c, p, h, d]
    wkT = qkv_w[idx[1], :].T.reshape(EC, 128, H, D)
    wqk_l = np.empty((H // 2, 128, EC, 4, D), dtype=np.float32)
    for pr in range(H // 2):
        wqk_l[pr, :, :, 0, :] = wqT[:, :, 2 * pr, :].transpose(1, 0, 2)
        wqk_l[pr, :, :, 1, :] = wkT[:, :, 2 * pr, :].transpose(1, 0, 2)
        wqk_l[pr, :, :, 2, :] = wqT[:, :, 2 * pr + 1, :].transpose(1, 0, 2)
        wqk_l[pr, :, :, 3, :] = wkT[:, :, 2 * pr + 1, :].transpose(1, 0, 2)
    wqk_l = np.ascontiguousarray(wqk_l.reshape(H // 2, 128, EC * 4 * D).astype(dtf))

    # wv: [128, EC*H*DP]; col c*H*DP + h*DP + d = qkv_w[idx2[h*D+d], c*128+p], pad 0
    wvT = qkv_w[idx[2], :].T.reshape(EC, 128, H, D)  # [c, p, h, d]
    wv_l = np.zeros((128, EC, H, VW), dtype=np.float32)
    wv_l[:, :, :, 1:D + 1] = wvT.transpose(1, 0, 2, 3)
    wv_l = np.ascontiguousarray(wv_l.reshape(128, EC * H * VW).astype(dtf))

    # vb: [128, H*DP] broadcast v-bias + ones column at d=D
    vb_row = np.zeros((H, VW), dtype=np.float32)
    vb_row[:, 1:D + 1] = qkv_b[idx[2]].reshape(H, D)
    vb_row[:, 0] = 1.0
    vb_l = np.ascontiguousarray(np.broadcast_to(vb_row.reshape(1, H * VW), (128, H * VW)))

    # bqk: [128, 3*H/2]; col p*3+m = per-partition bias for packed chunk m
    bq = qkv_b[idx[0]].reshape(H, D)
    bk = qkv_b[idx[1]].reshape(H, D)
    bqk_l = np.zeros((128, (H // 2) * 3), dtype=np.float32)
    for pr in range(H // 2):
        bqk_l[0:96, pr * 3 + 0] = bq[2 * pr]
        bqk_l[96:128, pr * 3 + 0] = bk[2 * pr][0:32]
        bqk_l[0:64, pr * 3 + 1] = bk[2 * pr][32:96]
        bqk_l[64:128, pr * 3 + 1] = bq[2 * pr + 1][0:64]
        bqk_l[0:32, pr * 3 + 2] = bq[2 * pr + 1][64:96]
        bqk_l[32:128, pr * 3 + 2] = bk[2 * pr + 1][0:96]

    # pw: [128, H*E]; pw_l[p, h*E+e] = proj_w[e, h*D+dd] for p=dd<D else 0
    pw_l = np.zeros((128, H, E), dtype=np.float32)
    pw_l[1:D + 1, :, :] = proj_w.reshape(E, H, D).transpose(2, 1, 0)
    pw_l = np.ascontiguousarray(pw_l.reshape(128, H * E).astype(ml_dtypes.bfloat16))

    pb_l = np.ascontiguousarray(np.broadcast_to(proj_b.reshape(1, E), (128, E)))

    # x^T per batch in sbuf layout: [B, 128, EC*N]; [b, p, c*N+n] = x[b, n, c*128+p]
    xt_all = np.ascontiguousarray(
        x.reshape(B, N, EC, 128).transpose(0, 3, 2, 1).reshape(B, 128, EC * N)
    ).astype(dtf)

    in_maps = []
    for core in range(NCORES):
        xt_core = np.ascontiguousarray(
            xt_all[core * BPC:(core + 1) * BPC]
        )
        in_maps.append(
            {
                "xt": xt_core,
                "wqk": wqk_l,
                "wv": wv_l,
                "vb": vb_l,
                "bqk": bqk_l,
                "pw": pw_l,
                "pb": pb_l,
            }
        )
    return in_maps


def run(inputs, trace=False):
    from concourse.bass_utils import run_bass_kernel_spmd

    nc = get_nc()
    in_maps = _prep_inputs(**inputs)
    res = run_bass_kernel_spmd(
        nc, in_maps, core_ids=list(range(NCORES)), trace=trace
    )
    out = np.concatenate([res.results[c]["out"] for c in range(NCORES)], axis=0)
    return out, res


def kernel(**inputs) -> np.ndarray:
    out, _ = run(inputs, trace=False)
    return out
